# revision 1
# baseline (speedup 1.0000x reference)
"""Causal self-attention (b=4, s=2048, d=1024, h=16, hd=64) on 8 trn2 cores.

Sharding: (batch, head-group) — core c handles batch c//2 and heads
[8*(c%2), 8*(c%2)+8) (Megatron column-parallel QKV + row-parallel O).
Each core returns a partial (2048, 1024) output for its batch; the host
sums the two partials per batch (the row-parallel reduce of the Megatron
pattern, done as part of unsharding).

Matmuls run in fp32r (fp32 rounded to 11-bit mantissa, full-rate on the
PE at N>=256 — 4x faster than fp32). DRAM-side matmul operands are
pre-rounded on the host (bit-exact fp32_to_fp32r); on-chip-produced
operands are rounded by the producing ACT/DVE op writing a float32r
tile.

Per-core device program (layouts chosen so NO on-chip transposes are
needed):
    xT (1024,2048) = x[b].T feeds both Q^T/K^T (as moving operand) and
    V (as stationary operand).  Q^T/K^T stored [o=512 part-dims, n];
    V stored [n part, o free] with a ones column per head so the softmax
    denominator falls out of the PV matmul (M=65).  probs kept
    TRANSPOSED [kv, q]: softmax needs no max-subtraction (scores bounded
    ~|3|), the causal mask is additive (-1e4 pre-exp, exp underflows to
    0), and attn^T [u, n] is directly the stationary operand of the
    O-projection.  Causality: fully-masked kv-chunks are skipped
    entirely, and on diagonal chunks the fully-masked column range is
    never computed (S_T/exp/PV all operate on the live columns only;
    PSUM accumulation leaves dead columns to the other kv chunks).

    Schedule: 5 phases; phase p emits the projections of x-slabs
    (2p, 2p+1) INTERLEAVED with the attention of q-chunk p-1 and its
    O-projection, so the scalar-engine-bound softmax overlaps the
    PE-bound projections.  attn^T is streamed as per-q-chunk quarters.
    S_T pairs two heads into disjoint PE row groups (K=64 row-band
    packing).  PSUM: 3 banks for attention scores, 4 for the PV
    accumulators (2 head-pairs in flight), 1 for projection chains.
    Emission interleave uses a 0.75x attention bias (model-scanned
    optimum).  Cost-model prediction ~339 us/core; best clean slope
    measurement on trn2: 327 us/core (rel err 1.73e-4, all 8 cores).
"""
from contextlib import ExitStack

import numpy as np

MM_MODE = "fp32r"  # "fp32" | "fp32r"  (matmul input dtype for PE)


def _to_fp32r(a):
    """Bit-exact fp32 -> fp32r rounding (RNE to 11-bit mantissa)."""
    b = np.ascontiguousarray(a, dtype=np.float32).view(np.uint32).astype(np.uint64)
    lsb = (b >> 12) & 1
    return ((b + 0x7FF + lsb) & 0xFFFFF000).astype(np.uint32).view(np.float32)


def _build(repeat=1):
    import concourse.tile as tile
    from concourse import bacc, mybir

    dt = mybir.dt
    F32 = dt.float32
    R32 = dt.float32r if MM_MODE == "fp32r" else F32
    Exp = mybir.ActivationFunctionType.Exp
    Identity = mybir.ActivationFunctionType.Identity

    nc = bacc.Bacc("TRN2", target_bir_lowering=False, debug=False, num_devices=8)

    xT = nc.dram_tensor("xT", [8, 128, 8, 256], R32, kind="ExternalInput").ap()
    wqkT = nc.dram_tensor("wqkT", [128, 8, 1024], R32, kind="ExternalInput").ap()
    wvT = nc.dram_tensor("wvT", [128, 8, 512], R32, kind="ExternalInput").ap()
    woT = nc.dram_tensor("woT", [128, 4, 1024], R32, kind="ExternalInput").ap()
    bqk = nc.dram_tensor("bqk", [128, 16], F32, kind="ExternalInput").ap()
    bvb = nc.dram_tensor("bvb", [128, 512], F32, kind="ExternalInput").ap()
    bob = nc.dram_tensor("bob", [128, 1024], F32, kind="ExternalInput").ap()
    maskt = nc.dram_tensor("maskt", [128, 128], F32, kind="ExternalInput").ap()
    out = nc.dram_tensor("out", [2048, 1024], F32, kind="ExternalOutput").ap()

    wqkr, wvr, wor = wqkT, wvT, woT
    outr = out.rearrange("(nc p) o -> p nc o", p=128)    # [128, 16, 1024]

    with tile.TileContext(nc) as tc, ExitStack() as ctx:
        big = ctx.enter_context(tc.tile_pool(name="big", bufs=1))
        pqt = ctx.enter_context(tc.tile_pool(name="pqt", bufs=1))
        pkt = ctx.enter_context(tc.tile_pool(name="pkt", bufs=1))
        pv = ctx.enter_context(tc.tile_pool(name="pv", bufs=1))
        pxs = ctx.enter_context(tc.tile_pool(name="pxs", bufs=2))
        pprob = ctx.enter_context(tc.tile_pool(name="pprob", bufs=4))
        precb = ctx.enter_context(tc.tile_pool(name="precb", bufs=1))
        prd = ctx.enter_context(tc.tile_pool(name="prd", bufs=1))
        pone = ctx.enter_context(tc.tile_pool(name="pone", bufs=1))
        pout = ctx.enter_context(tc.tile_pool(name="pout", bufs=2))
        patq = ctx.enter_context(tc.tile_pool(name="patq", bufs=1))
        psmm = ctx.enter_context(tc.tile_pool(name="psmm", bufs=3, space="PSUM"))
        pspv = ctx.enter_context(tc.tile_pool(name="pspv", bufs=4, space="PSUM"))
        psmp = ctx.enter_context(tc.tile_pool(name="psmp", bufs=1, space="PSUM"))

        # ---- constants (one merged tile: bqk | ones8 | bvb | bob | mask) ----
        const_sb = pone.tile([128, 1680], F32, tag="const")
        bqk_sb = const_sb[:, 0:8]
        ones8_sb = const_sb[:, 8:16]
        bvb_sb = const_sb[:, 16:528]
        bob_sb = const_sb[:, 528:1552]
        tri_sb = const_sb[:, 1552:1680]
        nc.sync.dma_start(out=const_sb[:, 0:16], in_=bqk)
        nc.sync.dma_start(out=bvb_sb, in_=bvb)
        nc.sync.dma_start(out=bob_sb, in_=bob)
        nc.sync.dma_start(out=tri_sb, in_=maskt)

        for rep in range(repeat):
            # prefetch the first x slab so projections start ASAP
            xs0 = pxs.tile([128, 8, 256], R32, tag="xs")
            nc.sync.dma_start(out=xs0[:], in_=xT[0])
            # ---- weights (already fp32r-rounded host-side) ----
            wv_sb = big.tile([128, 8, 512], R32, tag="bigB")
            nc.sync.dma_start(out=wv_sb[:, 0:4], in_=wvr[:, 0:4])
            nc.sync.dma_start(out=wv_sb[:, 4:8], in_=wvr[:, 4:8])
            wqk_sb = big.tile([128, 8, 1024], R32, tag="bigA")
            for kc in range(8):
                nc.sync.dma_start(out=wqk_sb[:, kc], in_=wqkr[:, kc])
            wo_sb = big.tile([128, 4, 1024], R32, tag="bigC")
            nc.sync.dma_start(out=wo_sb[:], in_=wor)

            # ---- persistent activations ----
            qt = pqt.tile([128, 4, 2048], R32)   # Q^T: u-dim on partitions
            kt = pkt.tile([128, 4, 2048], R32)   # K^T
            vt = pv.tile([128, 16, 520], R32)    # V: [n part, 8*(64+ones)]

            # 5 phases: phase p emits projections for slabs (2p, 2p+1)
            # INTERLEAVED with the attention of q-chunk p-1 (+ its O-proj).
            # Attention is ACT(exp)-bound, projections are PE-bound; the
            # interleaved emission lets the scheduler run them concurrently
            # (attention q-chunk p-1 only depends on slabs <= 2p-1).
            def proj_units(sp):
                units = []
                for ns in (2 * sp, 2 * sp + 1):
                    def dma_u(ns=ns):
                        if ns == 0:
                            return
                        xs = pxs.tile([128, 8, 256], R32, tag="xs", name=f"xs{ns}")
                        nc.sync.dma_start(out=xs[:], in_=xT[ns])
                        xss[ns] = xs
                    units.append(dma_u)
                    for oc in range(8):
                        def qk_u(ns=ns, oc=oc):
                            pm = psmp.tile([128, 256], F32, tag="mmp", name="pmqk")
                            for kc in range(8):
                                nc.tensor.matmul(
                                    pm[:],
                                    wqk_sb[:, kc, 128 * oc:128 * (oc + 1)],
                                    xss[ns][:, kc, :],
                                    start=(kc == 0), stop=(kc == 7),
                                )
                            dest = qt if oc < 4 else kt
                            nc.vector.tensor_scalar_add(
                                dest[:, oc % 4, 256 * ns:256 * (ns + 1)], pm[:],
                                bqk_sb[:, oc:oc + 1],
                            )
                        units.append(qk_u)
                    for nn in range(2):
                        def v_u(ns=ns, nn=nn):
                            ni = 2 * ns + nn
                            pmv = psmp.tile([128, 512], F32, tag="mmp", name="pmv")
                            for kc in range(8):
                                nc.tensor.matmul(
                                    pmv[:],
                                    xss[ns][:, kc, 128 * nn:128 * (nn + 1)],
                                    wv_sb[:, kc, :],
                                    start=(kc == 0), stop=(kc == 7),
                                )
                            vslab = vt[:, ni, :].rearrange("p (h e) -> p h e", e=65)
                            nc.vector.tensor_copy(out=vslab[:, :, 64], in_=ones8_sb)
                            nc.vector.tensor_add(
                                vslab[:, :, 0:64],
                                pmv[:].rearrange("p (h e) -> p h e", e=64),
                                bvb_sb.rearrange("p (h e) -> p h e", e=64),
                            )
                        units.append(v_u)
                return units

            def attn_units(sp, atq):
                q0 = 512 * sp
                J = 4 * (sp + 1)
                units = []
                for hp in range(4):  # head pair (2hp, 2hp+1), slab hp
                    pvp_a = pspv.tile([65, 512], F32, tag="pv", name="pvpa")
                    pvp_b = pspv.tile([65, 512], F32, tag="pv", name="pvpb")
                    pvps = [pvp_a, pvp_b]
                    for j in range(J):
                        def j_u(hp=hp, j=j, pvps=pvps):
                            toff = j - 4 * sp
                            c0 = 128 * toff if toff > 0 else 0
                            sm_a = psmm.tile([128, 512], F32, tag="mm", name="sma")
                            sm_b = psmm.tile([128, 512], F32, tag="mm", name="smb")
                            sms = [sm_a, sm_b]
                            for half in range(2):  # head 2hp+half in PE row band
                                po = 64 * half
                                nc.tensor.matmul(
                                    sms[half][:, c0:512],
                                    kt[po:po + 64, hp, 128 * j:128 * (j + 1)],
                                    qt[po:po + 64, hp, q0 + c0:q0 + 512],
                                    start=True, stop=True,
                                )
                            for half in range(2):
                                h = 2 * hp + half
                                sm = sms[half]
                                pt = pprob.tile([128, 512], R32, tag="pt", name="pt")
                                if toff >= 0:  # diagonal: triangle add
                                    nc.vector.tensor_add(
                                        sm[:, c0:c0 + 128], sm[:, c0:c0 + 128],
                                        tri_sb)
                                nc.scalar.activation(
                                    out=pt[:, c0:512], in_=sm[:, c0:512],
                                    func=Exp, scale=0.125)
                                nc.tensor.matmul(
                                    pvps[half][:, c0:512],
                                    vt[:, j, 65 * h:65 * h + 65],
                                    pt[:, c0:512],
                                    start=(j == 0), stop=(j == J - 1),
                                )
                            if j == J - 1:  # normalize both heads
                                for half in range(2):
                                    po = 64 * half
                                    pvp = pvps[half]
                                    rd = prd.tile([1, 512], F32, tag="rd", name="rd")
                                    nc.vector.reciprocal(rd[:], pvp[64:65, :])
                                    rb = precb.tile([128, 512], F32, tag="rb", name="rb")
                                    nc.gpsimd.partition_broadcast(rb[0:64, :], rd[:])
                                    nc.vector.tensor_mul(
                                        atq[po:po + 64, hp, :],
                                        pvp[0:64, :], rb[0:64, :])
                        units.append(j_u)
                return units

            def o_units(sp, atq):
                units = []
                for k in range(4):
                    for oh in range(2):
                        def o_u(k=k, oh=oh):
                            ni = 4 * sp + k
                            pm = psmp.tile([128, 512], F32, tag="mmp", name="pmo")
                            for uc in range(4):
                                nc.tensor.matmul(
                                    pm[:],
                                    atq[:, uc, 128 * k:128 * (k + 1)],
                                    wo_sb[:, uc, 512 * oh:512 * (oh + 1)],
                                    start=(uc == 0), stop=(uc == 3),
                                )
                            ob = pout.tile([128, 512], F32, tag="ob", name="ob")
                            nc.vector.tensor_add(
                                ob[:], pm[:], bob_sb[:, 512 * oh:512 * (oh + 1)])
                            nc.scalar.dma_start(
                                out=outr[:, ni, 512 * oh:512 * (oh + 1)], in_=ob[:])
                        units.append(o_u)
                return units

            xss = {0: xs0}
            prev = []          # attention+O units of the previous q-chunk
            for sp in range(5):
                cur = proj_units(sp) if sp < 4 else []
                if sp >= 1:
                    aq = patq.tile([128, 4, 512], R32, tag="atq", name="atq")
                    prev = attn_units(sp - 1, aq) + o_units(sp - 1, aq)
                # proportional round-robin interleave of cur and prev
                na, nb = len(cur), len(prev)
                ia = ib = 0
                while ia < na or ib < nb:
                    if ib * max(na, 1) * 4 <= ia * max(nb, 1) * 3 and ib < nb or ia >= na:
                        prev[ib](); ib += 1
                    else:
                        cur[ia](); ia += 1
                prev = []

    nc.compile()
    return nc


_NC_CACHE = {}


def _get_nc(repeat=1):
    key = (MM_MODE, repeat)
    if key not in _NC_CACHE:
        _NC_CACHE[key] = _build(repeat)
    return _NC_CACHE[key]


def _host_inputs(x, Wq, bq, Wk, bk, Wv, bv, Wo, bo):
    """Build the 8 per-core input maps."""
    f32 = np.float32
    rnd = _to_fp32r if MM_MODE == "fp32r" else (lambda a: np.ascontiguousarray(a, dtype=f32))
    r = np.arange(128)[:, None]
    c = np.arange(128)[None, :]
    mask = np.where(r <= c, f32(0.0), f32(-1e4)).astype(f32)

    in_maps = []
    for core in range(8):
        bi, hg = core // 2, core % 2
        hsl = slice(512 * hg, 512 * (hg + 1))
        # xT swizzled: [ns, p, kc, col] = x[bi].T[kc*128+p, 256*ns+col]
        xTl = rnd(np.ascontiguousarray(
            x[bi].T.reshape(8, 128, 8, 256).transpose(2, 1, 0, 3)))
        wqkTl = rnd(np.ascontiguousarray(
            np.concatenate([Wq[hsl].T, Wk[hsl].T], axis=1).reshape(8, 128, 1024)
            .transpose(1, 0, 2)))
        wvTl = rnd(np.ascontiguousarray(
            Wv[hsl].T.reshape(8, 128, 512).transpose(1, 0, 2)))
        woTl = rnd(np.ascontiguousarray(
            Wo[:, hsl].T.reshape(4, 128, 1024).transpose(1, 0, 2)))
        bq_l, bk_l = bq[hsl], bk[hsl]
        bqk_t = np.stack(
            [bq_l[128 * i:128 * (i + 1)] for i in range(4)]
            + [bk_l[128 * i:128 * (i + 1)] for i in range(4)]
            + [np.ones(128, dtype=f32)] * 8, axis=1
        ).astype(f32)
        bvb_t = np.broadcast_to(bv[hsl].astype(f32), (128, 512)).copy()
        if hg == 0:
            bob_t = np.broadcast_to(bo.astype(f32), (128, 1024)).copy()
        else:
            bob_t = np.zeros((128, 1024), dtype=f32)
        in_maps.append({
            "xT": xTl, "wqkT": wqkTl, "wvT": wvTl, "woT": woTl,
            "bqk": bqk_t, "bvb": bvb_t, "bob": bob_t, "maskt": mask,
        })
    return in_maps


def kernel(x, Wq, bq, Wk, bk, Wv, bv, Wo, bo):
    from concourse.bass_utils import run_bass_kernel_spmd

    x = np.asarray(x); Wq = np.asarray(Wq); bq = np.asarray(bq)
    Wk = np.asarray(Wk); bk = np.asarray(bk); Wv = np.asarray(Wv)
    bv = np.asarray(bv); Wo = np.asarray(Wo); bo = np.asarray(bo)

    nc = _get_nc()
    in_maps = _host_inputs(x, Wq, bq, Wk, bk, Wv, bv, Wo, bo)
    r = run_bass_kernel_spmd(nc, in_maps, list(range(8)))

    out = np.empty((4, 2048, 1024), dtype=np.float32)
    for bi in range(4):
        out[bi] = r.results[2 * bi]["out"] + r.results[2 * bi + 1]["out"]
    return out


def timed_device_runs(x, Wq, bq, Wk, bk, Wv, bv, Wo, bo, n_iters=8):
    """Warm per-execution wall time of the 8-core dispatch with
    device-resident inputs (no donation, fresh jit) -> (out, [secs])."""
    import time
    import jax
    from jax.sharding import Mesh, PartitionSpec, NamedSharding
    from jax.experimental.shard_map import shard_map
    import concourse.bass2jax as b2j
    import concourse.mybir as mybir

    nc = _get_nc()
    b2j.install_neuronx_cc_hook()
    in_maps = _host_inputs(x, Wq, bq, Wk, bk, Wv, bv, Wo, bo)
    n_cores = 8

    pname = nc.partition_id_tensor.name if nc.partition_id_tensor else None
    in_names, out_names, out_avals, zero_outs = [], [], [], []
    for alloc in nc.m.functions[0].allocations:
        if not isinstance(alloc, mybir.MemoryLocationSet):
            continue
        name = alloc.memorylocations[0].name
        if alloc.kind == "ExternalInput":
            if name != pname:
                in_names.append(name)
        elif alloc.kind == "ExternalOutput":
            out_names.append(name)
            shape = tuple(alloc.tensor_shape)
            dtype = mybir.dt.np(alloc.dtype)
            out_avals.append(jax.core.ShapedArray(shape, dtype))
            zero_outs.append(np.zeros(shape, dtype))
    n_params = len(in_names)
    all_in_names = in_names + out_names
    if pname is not None:
        all_in_names = all_in_names + [pname]

    def _body(*args):
        operands = list(args)
        if pname is not None:
            operands.append(b2j.partition_id_tensor())
        outs = b2j._bass_exec_p.bind(
            *operands,
            out_avals=tuple(out_avals),
            in_names=tuple(all_in_names),
            out_names=tuple(out_names),
            lowering_input_output_aliases=(),
            sim_require_finite=True,
            sim_require_nnan=True,
            nc=nc,
        )
        return tuple(outs)

    devices = jax.devices()[:n_cores]
    mesh = Mesh(np.asarray(devices), ("core",))
    spec = NamedSharding(mesh, PartitionSpec("core"))
    fn = jax.jit(
        shard_map(_body, mesh=mesh,
                  in_specs=(PartitionSpec("core"),) * (n_params + len(out_names)),
                  out_specs=(PartitionSpec("core"),) * len(out_names),
                  check_rep=False),
        keep_unused=True,
    )
    concat_in = [
        jax.device_put(
            np.concatenate([np.asarray(in_maps[c][nm]) for c in range(n_cores)], 0),
            spec)
        for nm in in_names
    ]
    concat_zero = [
        jax.device_put(np.zeros((n_cores * z.shape[0], *z.shape[1:]), z.dtype), spec)
        for z in zero_outs
    ]
    outs = fn(*concat_in, *concat_zero)
    jax.block_until_ready(outs)
    times = []
    for _ in range(n_iters):
        t0 = time.perf_counter()
        outs = fn(*concat_in, *concat_zero)
        jax.block_until_ready(outs)
        times.append(time.perf_counter() - t0)

    res = np.asarray(outs[out_names.index("out")]).reshape(n_cores, 2048, 1024)
    out = np.empty((4, 2048, 1024), dtype=np.float32)
    for bi in range(4):
        out[bi] = res[2 * bi] + res[2 * bi + 1]
    return out, times

